# revision 27
# baseline (speedup 1.0000x reference)
"""LocalAttention (banded) Trainium2 kernel, 8-core SPMD.

Problem: B=2, S=2048, H=1024, nh=16, hd=64, window=256 (half_w=128).
  q = x@Wq+bq ; k = x@Wk+bk ; v = x@Wv+bv  (per-head dim 64)
  scores = q.k/8 masked to |i-j|<=128 ; out = softmax(scores)@v @ Wo + bo

Sharding: core c -> batch c//4, token block (c%4)*512..+512.  Each core
receives a zero-padded 768-token slice of x (128-token halo each side,
recomputed locally; no cross-core communication).

On-chip: fully "transposed" layout (features on partitions).  Scores are
computed transposed (S^T = K @ Q^T) in kc-major groups (wide moving
operands).  The softmax denominator is fused into the PV matmul by
appending a ones-column to each V tile; PV output is q-major so the
normalization is a per-partition broadcast multiply (no cross-partition
data movement anywhere).  The normalized q-major output chunk is
transposed back to feature-major via DMA-transpose on the sync/scalar
queues.  K/V projections and attention are interleaved on the tensor
queue; output projection is pipelined in q-halves at the end.
1/sqrt(hd) is folded into Wq/bq and bv@Wo+bo into a single output bias
on the host.
"""

import sys

if "/opt/trn_rl_repo" not in sys.path:
    sys.path.insert(0, "/opt/trn_rl_repo")

import numpy as np
import ml_dtypes

B, S, H = 2, 2048, 1024
NH, HD = 16, 64
HALF_W = 128
NCORES = 8
BLK = 512          # owned tokens per core
PAD = 768          # owned + 2*128 halo
NQB = 4            # q-blocks of 128 per core
NKC = 6            # padded-local k chunks of 128
BF16 = ml_dtypes.bfloat16

# kc-major score/e-tile layout: for key chunk kc, the valid q-blocks are
# qb in [KC_QB0[kc], KC_QB0[kc]+KC_NQB[kc]).  Column offset of chunk
# (kc, qb) inside the per-sub 1536-col half is
#   KC_OFF[kc] + 128*(qb - KC_QB0[kc]).
KC_QB0 = [0, 0, 0, 1, 2, 3]
KC_NQB = [1, 2, 3, 3, 2, 1]
KC_OFF = [0, 128, 384, 768, 1152, 1408]
GROUPS = [[0, 1], [2], [3], [4, 5]]     # kc's per 384-col score group
GLOC = {0: 0, 1: 128, 2: 0, 3: 0, 4: 0, 5: 256}  # chunk offset in group

_COMPILED = None


def _build_core_inputs(x, Wq, bq, Wk, bk, Wv, bv, Wo, bo):
    """Host-side sharding / layout prep. Returns list of 8 in_maps."""
    x = np.asarray(x, np.float32)
    scale = 1.0 / np.sqrt(HD)
    wq_s = (np.asarray(Wq, np.float32) * scale).astype(BF16)
    wk_s = np.asarray(Wk, np.float32).astype(BF16)
    wv_s = np.asarray(Wv, np.float32).astype(BF16)
    wo_s = np.asarray(Wo, np.float32).astype(BF16)
    bq_s = (np.asarray(bq, np.float32) * scale)
    bk_s = np.asarray(bk, np.float32)
    # v-bias passes through attention unchanged (softmax rows sum to 1),
    # so it folds into the output bias: bo' = bo + bv @ Wo.
    bo_s = np.asarray(bo, np.float32) + np.asarray(bv, np.float32) @ np.asarray(Wo, np.float32)

    def as_pcols(vec):  # [1024] -> [128, 8] with [:, c] = vec[128c:128c+128]
        return np.ascontiguousarray(vec.reshape(8, 128).T, dtype=np.float32)

    bq_t, bk_t, bo_t = as_pcols(bq_s), as_pcols(bk_s), as_pcols(bo_s)

    in_maps = []
    for c in range(NCORES):
        b, blk = divmod(c, 4)
        t0 = blk * BLK
        lo, hi = t0 - HALF_W, t0 + BLK + HALF_W
        xp = np.zeros((PAD, H), np.float32)
        glo, ghi = max(lo, 0), min(hi, S)
        xp[glo - lo:ghi - lo] = x[b, glo:ghi]
        xT = np.ascontiguousarray(xp.T, dtype=BF16)  # [1024, 768]

        # kc-major mask: tile element [p, KC_OFF[kc]+128*(qb-qb0)+i]
        # guards key token lo+128*kc+p vs query token t0+128*qb+i.
        mask = np.zeros((128, 1536), BF16)
        p = np.arange(128)
        for kc in range(NKC):
            kg = lo + 128 * kc + p
            for j in range(KC_NQB[kc]):
                qb = KC_QB0[kc] + j
                qg = t0 + 128 * qb + p
                valid = (np.abs(kg[:, None] - qg[None, :]) <= HALF_W) & \
                        (kg[:, None] >= 0) & (kg[:, None] < S)
                col = KC_OFF[kc] + 128 * j
                mask[:, col:col + 128] = valid
        in_maps.append({
            "xT": xT,
            "wq": wq_s, "wk": wk_s, "wv": wv_s, "wo": wo_s,
            "bq_t": bq_t, "bk_t": bk_t, "bo_t": bo_t,
            "mask": mask,
            "ident": np.eye(128, dtype=np.float32).astype(BF16),
        })
    return in_maps


def _build_bass():
    import concourse.bass as bass
    import concourse.tile as tile
    from concourse import bacc, mybir
    from contextlib import ExitStack

    f32, bf16 = mybir.dt.float32, mybir.dt.bfloat16
    Id = mybir.ActivationFunctionType.Identity
    Exp = mybir.ActivationFunctionType.Exp

    nc = bacc.Bacc(None)
    d_xT = nc.declare_dram_parameter("xT", [H, PAD], bf16, isOutput=False)
    d_wq = nc.declare_dram_parameter("wq", [H, H], bf16, isOutput=False)
    d_wk = nc.declare_dram_parameter("wk", [H, H], bf16, isOutput=False)
    d_wv = nc.declare_dram_parameter("wv", [H, H], bf16, isOutput=False)
    d_wo = nc.declare_dram_parameter("wo", [H, H], bf16, isOutput=False)
    d_bq = nc.declare_dram_parameter("bq_t", [128, 8], f32, isOutput=False)
    d_bk = nc.declare_dram_parameter("bk_t", [128, 8], f32, isOutput=False)
    d_bo = nc.declare_dram_parameter("bo_t", [128, 8], f32, isOutput=False)
    d_mask = nc.declare_dram_parameter("mask", [128, 1536], bf16, isOutput=False)
    d_ident = nc.declare_dram_parameter("ident", [128, 128], bf16, isOutput=False)
    d_out = nc.declare_dram_parameter("out", [H, BLK], bf16, isOutput=True)

    with tile.TileContext(nc) as tc, ExitStack() as ctx:
        persist = ctx.enter_context(tc.tile_pool(name="persist", bufs=1))
        epool = ctx.enter_context(tc.tile_pool(name="epool", bufs=8))
        smallp = ctx.enter_context(tc.tile_pool(name="smallp", bufs=3))
        ytp = ctx.enter_context(tc.tile_pool(name="ytp", bufs=2))
        ps_proj = ctx.enter_context(tc.tile_pool(name="ps_proj", bufs=2, space="PSUM"))
        ps_s = ctx.enter_context(tc.tile_pool(name="ps_s", bufs=2, space="PSUM"))
        ps_po = ctx.enter_context(tc.tile_pool(name="ps_po", bufs=2, space="PSUM"))

        sb_xT = [persist.tile([128, PAD], bf16, name=f"xT{h}", tag=f"xT{h}") for h in range(8)]
        sb_wq = [persist.tile([128, H], bf16, name=f"wq{h}", tag=f"wq{h}") for h in range(8)]
        sb_wk = [persist.tile([128, H], bf16, name=f"wk{h}", tag=f"wk{h}") for h in range(8)]
        sb_wv = [persist.tile([128, H], bf16, name=f"wv{h}", tag=f"wv{h}") for h in range(8)]
        sb_wo = [persist.tile([128, H], bf16, name=f"wo{h}", tag=f"wo{h}") for h in range(8)]
        sb_qt = [persist.tile([128, BLK], bf16, name=f"qt{c}", tag=f"qt{c}") for c in range(8)]
        sb_kt = [persist.tile([128, PAD], bf16, name=f"kt{c}", tag=f"kt{c}") for c in range(8)]
        # v token-major, per head-pair column groups of 130:
        #   [64 f sub0][one][64 f sub1][one]
        sb_v = [persist.tile([128, 1040], bf16, name=f"v{t}", tag=f"v{t}") for t in range(6)]
        sb_oc = [persist.tile([128, BLK], bf16, name=f"oc{c}", tag=f"oc{c}") for c in range(8)]
        sb_mask = persist.tile([128, 1536], bf16, name="mask", tag="mask")
        sb_bq = persist.tile([128, 8], f32, name="bq", tag="bq")
        sb_bk = persist.tile([128, 8], f32, name="bk", tag="bk")
        sb_bo = persist.tile([128, 8], f32, name="bo", tag="bo")
        sb_ident = persist.tile([128, 128], bf16, name="ident", tag="ident")

        # ---- DMA issue --------------------------------------------------
        # The xT/wq chunks gate the first matmuls: put them first on the
        # two fast HWDGE queues (sync, scalar).  gpsimd (software DGE,
        # slower start) gets the later-needed wk/mask/wv.
        for h in range(4):
            nc.sync.dma_start(sb_xT[h][:], d_xT[128 * h:128 * (h + 1), :])
            nc.sync.dma_start(sb_wq[h][:], d_wq[128 * h:128 * (h + 1), :])
        for h in range(4, 8):
            nc.scalar.dma_start(sb_xT[h][:], d_xT[128 * h:128 * (h + 1), :])
            nc.scalar.dma_start(sb_wq[h][:], d_wq[128 * h:128 * (h + 1), :])
        nc.scalar.dma_start(sb_bq[:], d_bq[:])
        nc.scalar.dma_start(sb_bk[:], d_bk[:])
        nc.scalar.dma_start(sb_bo[:], d_bo[:])
        nc.scalar.dma_start(sb_ident[:], d_ident[:])
        for h in range(8):
            nc.gpsimd.dma_start(sb_wk[h][:], d_wk[128 * h:128 * (h + 1), :])
        nc.gpsimd.dma_start(sb_mask[:], d_mask[:])
        for h in range(8):
            nc.sync.dma_start(sb_wo[h][:], d_wo[128 * h:128 * (h + 1), :])
        for h in range(8):
            nc.gpsimd.dma_start(sb_wv[h][:], d_wv[128 * h:128 * (h + 1), :])

        # ones columns of the v tiles (cols 130c+64 and 130c+129)
        for t in range(6):
            vv = sb_v[t][:]
            ones_ap = bass.AP(tensor=vv.tensor, offset=vv.offset + 64,
                              ap=[vv.ap[0], [130, 8], [65, 2]])
            nc.vector.memset(ones_ap, 1.0)

        # ---- compute emitters -------------------------------------------
        def qproj(c):
            ps = ps_proj.tile([128, BLK], f32, name="psq", tag="psq")
            for h in range(8):
                nc.tensor.matmul(ps[:], sb_wq[h][:, 128 * c:128 * (c + 1)],
                                 sb_xT[h][:, 128:128 + BLK],
                                 start=(h == 0), stop=(h == 7))
            nc.scalar.activation(sb_qt[c][:], ps[:], Id, bias=sb_bq[:, c:c + 1])

        def kproj(c):
            for half in range(2):
                ps = ps_proj.tile([128, BLK], f32, name="psk", tag="psq")
                pv_ = ps[:, 0:384]
                for h in range(8):
                    nc.tensor.matmul(pv_, sb_wk[h][:, 128 * c:128 * (c + 1)],
                                     sb_xT[h][:, 384 * half:384 * (half + 1)],
                                     start=(h == 0), stop=(h == 7))
                nc.vector.tensor_scalar_add(
                    sb_kt[c][:, 384 * half:384 * (half + 1)], pv_,
                    sb_bk[:, c:c + 1])

        def vproj(t, half):
            ps = ps_proj.tile([128, BLK], f32, name="psv", tag="psq")
            for h in range(8):
                nc.tensor.matmul(ps[:], sb_xT[h][:, 128 * t:128 * (t + 1)],
                                 sb_wv[h][:, 512 * half:512 * (half + 1)],
                                 start=(h == 0), stop=(h == 7))
            vv = sb_v[t][:]
            dst = bass.AP(tensor=vv.tensor, offset=vv.offset + 520 * half,
                          ap=[vv.ap[0], [130, 4], [65, 2], [1, 64]])
            nc.vector.tensor_copy(dst, ps[:])

        def scores(c, e):
            ee = e[:]
            mm_ = sb_mask[:]
            for g in range(4):
                psc = ps_s.tile([128, 1024], f32, name="psc", tag="psc")
                pp = psc[:]
                for sub in range(2):
                    for kc in GROUPS[g]:
                        qb0 = KC_QB0[kc]
                        W = 128 * KC_NQB[kc]
                        loc = 512 * sub + GLOC[kc]
                        nc.tensor.matmul(
                            psc[:, loc:loc + W],
                            sb_kt[c][64 * sub:64 * (sub + 1), 128 * kc:128 * (kc + 1)],
                            sb_qt[c][64 * sub:64 * (sub + 1), 128 * qb0:128 * qb0 + W])
                src = bass.AP(tensor=pp.tensor, offset=pp.offset,
                              ap=[pp.ap[0], [512, 2], [1, 384]])
                dst = bass.AP(tensor=ee.tensor, offset=ee.offset + 384 * g,
                              ap=[ee.ap[0], [1536, 2], [1, 384]])
                nc.scalar.activation(dst, src, Exp)
                # mask the two triangle chunks of this group (both subs),
                # alternating engines (e and mask are SBUF so gpsimd is legal)
                ev = bass.AP(tensor=ee.tensor, offset=ee.offset + 384 * g,
                             ap=[ee.ap[0], [1536, 2], [256, 2], [1, 128]])
                mv = bass.AP(tensor=mm_.tensor, offset=mm_.offset + 384 * g,
                             ap=[mm_.ap[0], [0, 2], [256, 2], [1, 128]])
                eng = nc.vector if (c + g) % 2 == 0 else nc.gpsimd
                eng.tensor_mul(ev, ev, mv)

        # PE transposes of the normalized q-major chunk back to feature-
        # major are emitted one pv_half later (the normalize on vector must
        # land first; the lag keeps the in-order tensor queue stall-free).
        tr_pending = []

        def tr_flush():
            if tr_pending:
                tr_pending.pop(0)()

        def pv_half(c, half, e):
            # po layout: [128 q, 512]: region (j, sub) at 130*j + 65*sub;
            # 65 cols = 64 features + denominator.  half covers qb 2h..2h+1.
            ocq = smallp.tile([128, 256], bf16, name="ocq", tag="ocq")
            po = ps_po.tile([128, BLK], f32, name="po", tag="po")
            pp = po[:]
            for j in range(2):
                qb = 2 * half + j
                for sub in range(2):
                    for r in range(3):
                        kc = qb + r
                        ecol = 1536 * sub + KC_OFF[kc] + 128 * (qb - KC_QB0[kc])
                        nc.tensor.matmul(
                            po[:, 130 * j + 65 * sub:130 * j + 65 * (sub + 1)],
                            e[:, ecol:ecol + 128],
                            sb_v[kc][:, 130 * c + 65 * sub:130 * c + 65 * (sub + 1)],
                            start=(r == 0), stop=(r == 2))
            rc = smallp.tile([128, 4], f32, name="rc", tag="rc")
            dsrc = bass.AP(tensor=pp.tensor, offset=pp.offset + 64,
                           ap=[pp.ap[0], [65, 4]])
            nc.vector.reciprocal(rc[:], dsrc)
            for sub in range(2):
                src = bass.AP(tensor=pp.tensor, offset=pp.offset + 65 * sub,
                              ap=[pp.ap[0], [130, 2], [1, 64]])
                rv = bass.AP(tensor=rc.tensor, offset=rc[:].offset + sub,
                             ap=[rc[:].ap[0], [2, 2], [0, 64]])
                ov = bass.AP(tensor=ocq.tensor,
                             offset=ocq[:].offset + 64 * sub,
                             ap=[ocq[:].ap[0], [128, 2], [1, 64]])
                nc.vector.tensor_mul(ov, src, rv)

            def fin():
                # bf16 view-sized tile sharing the psc slot byte size
                tr = ps_s.tile([128, 2048], bf16, name="tr", tag="psc")
                for j in range(2):
                    nc.tensor.transpose(tr[:, 128 * j:128 * (j + 1)],
                                        ocq[:, 128 * j:128 * (j + 1)],
                                        sb_ident[:])
                for j in range(2):
                    qb = 2 * half + j
                    eng = nc.vector if j == 0 else nc.scalar
                    if eng is nc.scalar:
                        nc.scalar.activation(
                            sb_oc[c][:, 128 * qb:128 * (qb + 1)],
                            tr[:, 128 * j:128 * (j + 1)], Id)
                    else:
                        nc.vector.tensor_copy(
                            sb_oc[c][:, 128 * qb:128 * (qb + 1)],
                            tr[:, 128 * j:128 * (j + 1)])

            tr_pending.append(fin)

        def outproj(o, half):
            ps = ps_proj.tile([128, BLK], f32, name="psy", tag="psq")
            pw = ps[:, 0:256]
            for f in range(8):
                nc.tensor.matmul(pw, sb_wo[f][:, 128 * o:128 * (o + 1)],
                                 sb_oc[f][:, 256 * half:256 * (half + 1)],
                                 start=(f == 0), stop=(f == 7))
            yt = ytp.tile([128, 256], bf16, name="yt", tag="yt")
            if o % 2 == 0:
                nc.scalar.activation(yt[:], pw, Id, bias=sb_bo[:, o:o + 1])
            else:
                nc.vector.tensor_scalar_add(yt[:], pw, sb_bo[:, o:o + 1])
            sq = nc.sync if o % 2 == 0 else nc.gpsimd
            sq.dma_start(d_out[128 * o:128 * (o + 1), 256 * half:256 * (half + 1)], yt[:])

        # ---- emission schedule ------------------------------------------
        es = {}

        def s(c):
            es[c] = epool.tile([128, 3072], bf16, name="e", tag="e")
            scores(c, es[c])

        for c in range(8):
            qproj(c)
        for c in range(8):
            kproj(c)
        s(0)
        vproj(0, 0); vproj(0, 1)
        s(1)
        vproj(1, 0); vproj(1, 1)
        s(2)
        vproj(2, 0); vproj(2, 1)
        s(3)
        vproj(3, 0); vproj(3, 1)
        s(4)
        pv_half(0, 0, es[0])
        vproj(4, 0); vproj(4, 1)
        tr_flush()
        s(5)
        pv_half(1, 0, es[1])
        vproj(5, 0); vproj(5, 1)
        tr_flush()
        s(6)
        pv_half(0, 1, es[0])
        tr_flush()
        pv_half(2, 0, es[2])
        s(7)
        tr_flush()
        pv_half(1, 1, es[1])
        tr_flush()
        pv_half(3, 0, es[3])
        for c in range(4, 8):
            tr_flush()
            pv_half(c - 2, 1, es[c - 2])
            tr_flush()
            pv_half(c, 0, es[c])
        tr_flush()
        pv_half(6, 1, es[6])
        tr_flush()
        pv_half(7, 1, es[7])
        tr_flush()
        tr_flush()
        for half in range(2):
            for o in range(8):
                outproj(o, half)

    nc.compile()
    return nc


def _get_compiled():
    global _COMPILED
    if _COMPILED is None:
        _COMPILED = _build_bass()
    return _COMPILED


def kernel(x, Wq, bq, Wk, bk, Wv, bv, Wo, bo, _trace=False):
    from concourse.bass_utils import run_bass_kernel_spmd

    in_maps = _build_core_inputs(x, Wq, bq, Wk, bk, Wv, bv, Wo, bo)
    nc = _get_compiled()
    res = run_bass_kernel_spmd(nc, in_maps, core_ids=list(range(NCORES)),
                               trace=_trace)
    out = np.empty((B, S, H), np.float32)
    for c in range(NCORES):
        b, blk = divmod(c, 4)
        out[b, blk * BLK:(blk + 1) * BLK, :] = \
            res.results[c]["out"].astype(np.float32).T
    if _trace:
        return out, res
    return out


# revision 30
# speedup vs baseline: 1.1399x; 1.1399x over previous
"""LocalAttention (banded) Trainium2 kernel, 8-core SPMD.

Problem: B=2, S=2048, H=1024, nh=16, hd=64, window=256 (half_w=128).
  q = x@Wq+bq ; k = x@Wk+bk ; v = x@Wv+bv  (per-head dim 64)
  scores = q.k/8 masked to |i-j|<=128 ; out = softmax(scores)@v @ Wo + bo

Sharding: core c -> batch c//4, token block (c%4)*512..+512.  Each core
receives a zero-padded 768-token slice of x (128-token halo each side,
recomputed locally; no cross-core communication).

On-chip: fully "transposed" layout (features on partitions).  Scores are
computed transposed (S^T = K @ Q^T) in kc-major groups (wide moving
operands).  The softmax denominator is fused into the PV matmul by
appending a ones-column to each V tile; PV output is q-major so the
normalization is a per-partition broadcast multiply (no cross-partition
data movement anywhere).  The normalized q-major output chunk is
transposed back to feature-major via DMA-transpose on the sync/scalar
queues.  K/V projections and attention are interleaved on the tensor
queue; output projection is pipelined in q-halves at the end.
1/sqrt(hd) is folded into Wq/bq and bv@Wo+bo into a single output bias
on the host.
"""

import sys

if "/opt/trn_rl_repo" not in sys.path:
    sys.path.insert(0, "/opt/trn_rl_repo")

import numpy as np
import ml_dtypes

B, S, H = 2, 2048, 1024
NH, HD = 16, 64
HALF_W = 128
NCORES = 8
BLK = 512          # owned tokens per core
PAD = 768          # owned + 2*128 halo
NQB = 4            # q-blocks of 128 per core
NKC = 6            # padded-local k chunks of 128
BF16 = ml_dtypes.bfloat16

# kc-major score/e-tile layout: for key chunk kc, the valid q-blocks are
# qb in [KC_QB0[kc], KC_QB0[kc]+KC_NQB[kc]).  Column offset of chunk
# (kc, qb) inside the per-sub 1536-col half is
#   KC_OFF[kc] + 128*(qb - KC_QB0[kc]).
KC_QB0 = [0, 0, 0, 1, 2, 3]
KC_NQB = [1, 2, 3, 3, 2, 1]
KC_OFF = [0, 128, 384, 768, 1152, 1408]
GROUPS = [[0, 1], [2], [3], [4, 5]]     # kc's per 384-col score group
GLOC = {0: 0, 1: 128, 2: 0, 3: 0, 4: 0, 5: 256}  # chunk offset in group

_COMPILED = None


def _build_core_inputs(x, Wq, bq, Wk, bk, Wv, bv, Wo, bo):
    """Host-side sharding / layout prep. Returns list of 8 in_maps."""
    x = np.asarray(x, np.float32)
    scale = 1.0 / np.sqrt(HD)
    wq_s = (np.asarray(Wq, np.float32) * scale).astype(BF16)
    wk_s = np.asarray(Wk, np.float32).astype(BF16)
    wv_s = np.asarray(Wv, np.float32).astype(BF16)
    wo_s = np.asarray(Wo, np.float32).astype(BF16)
    bq_s = (np.asarray(bq, np.float32) * scale)
    bk_s = np.asarray(bk, np.float32)
    # v-bias passes through attention unchanged (softmax rows sum to 1),
    # so it folds into the output bias: bo' = bo + bv @ Wo.
    bo_s = np.asarray(bo, np.float32) + np.asarray(bv, np.float32) @ np.asarray(Wo, np.float32)

    def as_pcols(vec):  # [1024] -> [128, 8] with [:, c] = vec[128c:128c+128]
        return np.ascontiguousarray(vec.reshape(8, 128).T, dtype=np.float32)

    bq_t, bk_t, bo_t = as_pcols(bq_s), as_pcols(bk_s), as_pcols(bo_s)

    in_maps = []
    for c in range(NCORES):
        b, blk = divmod(c, 4)
        t0 = blk * BLK
        lo, hi = t0 - HALF_W, t0 + BLK + HALF_W
        xp = np.zeros((PAD, H), np.float32)
        glo, ghi = max(lo, 0), min(hi, S)
        xp[glo - lo:ghi - lo] = x[b, glo:ghi]
        xT = np.ascontiguousarray(xp.T, dtype=BF16)  # [1024, 768]

        # kc-major mask: tile element [p, KC_OFF[kc]+128*(qb-qb0)+i]
        # guards key token lo+128*kc+p vs query token t0+128*qb+i.
        mask = np.zeros((128, 1536), BF16)
        p = np.arange(128)
        for kc in range(NKC):
            kg = lo + 128 * kc + p
            for j in range(KC_NQB[kc]):
                qb = KC_QB0[kc] + j
                qg = t0 + 128 * qb + p
                valid = (np.abs(kg[:, None] - qg[None, :]) <= HALF_W) & \
                        (kg[:, None] >= 0) & (kg[:, None] < S)
                col = KC_OFF[kc] + 128 * j
                mask[:, col:col + 128] = valid
        in_maps.append({
            "xT": xT,
            "wq": wq_s, "wk": wk_s, "wv": wv_s, "wo": wo_s,
            "bq_t": bq_t, "bk_t": bk_t, "bo_t": bo_t,
            "mask": mask,
            "ident": np.eye(128, dtype=np.float32).astype(BF16),
        })
    return in_maps


def _build_bass():
    import concourse.bass as bass
    import concourse.tile as tile
    from concourse import bacc, mybir
    from contextlib import ExitStack

    f32, bf16 = mybir.dt.float32, mybir.dt.bfloat16
    Id = mybir.ActivationFunctionType.Identity
    Exp = mybir.ActivationFunctionType.Exp

    nc = bacc.Bacc(None)
    d_xT = nc.declare_dram_parameter("xT", [H, PAD], bf16, isOutput=False)
    d_wq = nc.declare_dram_parameter("wq", [H, H], bf16, isOutput=False)
    d_wk = nc.declare_dram_parameter("wk", [H, H], bf16, isOutput=False)
    d_wv = nc.declare_dram_parameter("wv", [H, H], bf16, isOutput=False)
    d_wo = nc.declare_dram_parameter("wo", [H, H], bf16, isOutput=False)
    d_bq = nc.declare_dram_parameter("bq_t", [128, 8], f32, isOutput=False)
    d_bk = nc.declare_dram_parameter("bk_t", [128, 8], f32, isOutput=False)
    d_bo = nc.declare_dram_parameter("bo_t", [128, 8], f32, isOutput=False)
    d_mask = nc.declare_dram_parameter("mask", [128, 1536], bf16, isOutput=False)
    d_ident = nc.declare_dram_parameter("ident", [128, 128], bf16, isOutput=False)
    d_out = nc.declare_dram_parameter("out", [H, BLK], bf16, isOutput=True)

    with tile.TileContext(nc) as tc, ExitStack() as ctx:
        persist = ctx.enter_context(tc.tile_pool(name="persist", bufs=1))
        epool = ctx.enter_context(tc.tile_pool(name="epool", bufs=8))
        smallp = ctx.enter_context(tc.tile_pool(name="smallp", bufs=3))
        ytp = ctx.enter_context(tc.tile_pool(name="ytp", bufs=4))
        ps_proj = ctx.enter_context(tc.tile_pool(name="ps_proj", bufs=2, space="PSUM"))
        ps_s = ctx.enter_context(tc.tile_pool(name="ps_s", bufs=2, space="PSUM"))
        ps_po = ctx.enter_context(tc.tile_pool(name="ps_po", bufs=2, space="PSUM"))

        sb_xT = [persist.tile([128, PAD], bf16, name=f"xT{h}", tag=f"xT{h}") for h in range(8)]
        sb_wq = [persist.tile([128, H], bf16, name=f"wq{h}", tag=f"wq{h}") for h in range(8)]
        sb_wk = [persist.tile([128, H], bf16, name=f"wk{h}", tag=f"wk{h}") for h in range(8)]
        sb_wv = [persist.tile([128, H], bf16, name=f"wv{h}", tag=f"wv{h}") for h in range(8)]
        sb_wo = [persist.tile([128, H], bf16, name=f"wo{h}", tag=f"wo{h}") for h in range(8)]
        sb_qt = [persist.tile([128, BLK], bf16, name=f"qt{c}", tag=f"qt{c}") for c in range(8)]
        sb_kt = [persist.tile([128, PAD], bf16, name=f"kt{c}", tag=f"kt{c}") for c in range(8)]
        # v token-major, per head-pair column groups of 130:
        #   [64 f sub0][one][64 f sub1][one]
        sb_v = [persist.tile([128, 1040], bf16, name=f"v{t}", tag=f"v{t}") for t in range(6)]
        sb_oc = [persist.tile([128, BLK], bf16, name=f"oc{c}", tag=f"oc{c}") for c in range(8)]
        sb_mask = persist.tile([128, 1536], bf16, name="mask", tag="mask")
        sb_bq = persist.tile([128, 8], f32, name="bq", tag="bq")
        sb_bk = persist.tile([128, 8], f32, name="bk", tag="bk")
        sb_bo = persist.tile([128, 8], f32, name="bo", tag="bo")
        sb_ident = persist.tile([128, 128], bf16, name="ident", tag="ident")

        # ---- DMA issue --------------------------------------------------
        # gpsimd's software DGE issues back-to-back (~650ns/256KB) while
        # the HWDGE rings (sync/scalar) only sustain ~1 transfer/1.3us, so
        # the start-gating xT/wq chunks go on gpsimd first, split with the
        # HWDGE queues.
        for h in range(4):
            nc.gpsimd.dma_start(sb_xT[h][:], d_xT[128 * h:128 * (h + 1), :])
            nc.gpsimd.dma_start(sb_wq[h][:], d_wq[128 * h:128 * (h + 1), :])
        for h in range(4, 6):
            nc.sync.dma_start(sb_xT[h][:], d_xT[128 * h:128 * (h + 1), :])
            nc.sync.dma_start(sb_wq[h][:], d_wq[128 * h:128 * (h + 1), :])
        for h in range(6, 8):
            nc.scalar.dma_start(sb_xT[h][:], d_xT[128 * h:128 * (h + 1), :])
            nc.scalar.dma_start(sb_wq[h][:], d_wq[128 * h:128 * (h + 1), :])
        nc.scalar.dma_start(sb_bq[:], d_bq[:])
        nc.scalar.dma_start(sb_bk[:], d_bk[:])
        nc.scalar.dma_start(sb_bo[:], d_bo[:])
        nc.scalar.dma_start(sb_ident[:], d_ident[:])
        for h in range(8):
            nc.gpsimd.dma_start(sb_wk[h][:], d_wk[128 * h:128 * (h + 1), :])
        for h in range(8):
            nc.sync.dma_start(sb_wo[h][:], d_wo[128 * h:128 * (h + 1), :])
        for h in range(8):
            nc.gpsimd.dma_start(sb_wv[h][:], d_wv[128 * h:128 * (h + 1), :])
        nc.gpsimd.dma_start(sb_mask[:], d_mask[:])

        # ones columns of the v tiles (cols 130c+64 and 130c+129)
        for t in range(6):
            vv = sb_v[t][:]
            ones_ap = bass.AP(tensor=vv.tensor, offset=vv.offset + 64,
                              ap=[vv.ap[0], [130, 8], [65, 2]])
            nc.vector.memset(ones_ap, 1.0)

        # ---- compute emitters -------------------------------------------
        def qproj(c):
            ps = ps_proj.tile([128, BLK], f32, name="psq", tag="psq")
            for h in range(8):
                nc.tensor.matmul(ps[:], sb_wq[h][:, 128 * c:128 * (c + 1)],
                                 sb_xT[h][:, 128:128 + BLK],
                                 start=(h == 0), stop=(h == 7))
            nc.scalar.activation(sb_qt[c][:], ps[:], Id, bias=sb_bq[:, c:c + 1])

        def kproj(c):
            for half in range(2):
                ps = ps_proj.tile([128, BLK], f32, name="psk", tag="psq")
                pv_ = ps[:, 0:384]
                for h in range(8):
                    nc.tensor.matmul(pv_, sb_wk[h][:, 128 * c:128 * (c + 1)],
                                     sb_xT[h][:, 384 * half:384 * (half + 1)],
                                     start=(h == 0), stop=(h == 7))
                nc.vector.tensor_scalar_add(
                    sb_kt[c][:, 384 * half:384 * (half + 1)], pv_,
                    sb_bk[:, c:c + 1])

        def vproj(t, half):
            ps = ps_proj.tile([128, BLK], f32, name="psv", tag="psq")
            for h in range(8):
                nc.tensor.matmul(ps[:], sb_xT[h][:, 128 * t:128 * (t + 1)],
                                 sb_wv[h][:, 512 * half:512 * (half + 1)],
                                 start=(h == 0), stop=(h == 7))
            vv = sb_v[t][:]
            dst = bass.AP(tensor=vv.tensor, offset=vv.offset + 520 * half,
                          ap=[vv.ap[0], [130, 4], [65, 2], [1, 64]])
            nc.vector.tensor_copy(dst, ps[:])

        def scores(c, e):
            ee = e[:]
            mm_ = sb_mask[:]
            for g in range(4):
                psc = ps_s.tile([128, 1024], f32, name="psc", tag="psc")
                pp = psc[:]
                for sub in range(2):
                    for kc in GROUPS[g]:
                        qb0 = KC_QB0[kc]
                        W = 128 * KC_NQB[kc]
                        loc = 512 * sub + GLOC[kc]
                        nc.tensor.matmul(
                            psc[:, loc:loc + W],
                            sb_kt[c][64 * sub:64 * (sub + 1), 128 * kc:128 * (kc + 1)],
                            sb_qt[c][64 * sub:64 * (sub + 1), 128 * qb0:128 * qb0 + W])
                src = bass.AP(tensor=pp.tensor, offset=pp.offset,
                              ap=[pp.ap[0], [512, 2], [1, 384]])
                dst = bass.AP(tensor=ee.tensor, offset=ee.offset + 384 * g,
                              ap=[ee.ap[0], [1536, 2], [1, 384]])
                nc.scalar.activation(dst, src, Exp)
                # mask the two triangle chunks of this group (both subs),
                # alternating engines (e and mask are SBUF so gpsimd is legal)
                ev = bass.AP(tensor=ee.tensor, offset=ee.offset + 384 * g,
                             ap=[ee.ap[0], [1536, 2], [256, 2], [1, 128]])
                mv = bass.AP(tensor=mm_.tensor, offset=mm_.offset + 384 * g,
                             ap=[mm_.ap[0], [0, 2], [256, 2], [1, 128]])
                eng = nc.vector if (c + g) % 2 == 0 else nc.gpsimd
                eng.tensor_mul(ev, ev, mv)

        # PE transposes of the normalized q-major chunk back to feature-
        # major are emitted one pv_half later (the normalize on vector must
        # land first; the lag keeps the in-order tensor queue stall-free).
        tr_pending = []

        def tr_flush():
            if tr_pending:
                tr_pending.pop(0)()

        def pv_half(c, half, e):
            # po layout: [128 q, 512]: region (j, sub) at 130*j + 65*sub;
            # 65 cols = 64 features + denominator.  half covers qb 2h..2h+1.
            ocq = smallp.tile([128, 256], bf16, name="ocq", tag="ocq")
            po = ps_po.tile([128, BLK], f32, name="po", tag="po")
            pp = po[:]
            for j in range(2):
                qb = 2 * half + j
                for sub in range(2):
                    for r in range(3):
                        kc = qb + r
                        ecol = 1536 * sub + KC_OFF[kc] + 128 * (qb - KC_QB0[kc])
                        nc.tensor.matmul(
                            po[:, 130 * j + 65 * sub:130 * j + 65 * (sub + 1)],
                            e[:, ecol:ecol + 128],
                            sb_v[kc][:, 130 * c + 65 * sub:130 * c + 65 * (sub + 1)],
                            start=(r == 0), stop=(r == 2))
            rc = smallp.tile([128, 4], f32, name="rc", tag="rc")
            dsrc = bass.AP(tensor=pp.tensor, offset=pp.offset + 64,
                           ap=[pp.ap[0], [65, 4]])
            nc.vector.reciprocal(rc[:], dsrc)
            for sub in range(2):
                src = bass.AP(tensor=pp.tensor, offset=pp.offset + 65 * sub,
                              ap=[pp.ap[0], [130, 2], [1, 64]])
                rv = bass.AP(tensor=rc.tensor, offset=rc[:].offset + sub,
                             ap=[rc[:].ap[0], [2, 2], [0, 64]])
                ov = bass.AP(tensor=ocq.tensor,
                             offset=ocq[:].offset + 64 * sub,
                             ap=[ocq[:].ap[0], [128, 2], [1, 64]])
                nc.vector.tensor_mul(ov, src, rv)

            def fin():
                # bf16 view-sized tile sharing the psc slot byte size
                tr = ps_s.tile([128, 2048], bf16, name="tr", tag="psc")
                for j in range(2):
                    nc.tensor.transpose(tr[:, 128 * j:128 * (j + 1)],
                                        ocq[:, 128 * j:128 * (j + 1)],
                                        sb_ident[:])
                for j in range(2):
                    qb = 2 * half + j
                    eng = nc.vector if j == 0 else nc.scalar
                    if eng is nc.scalar:
                        nc.scalar.activation(
                            sb_oc[c][:, 128 * qb:128 * (qb + 1)],
                            tr[:, 128 * j:128 * (j + 1)], Id)
                    else:
                        nc.vector.tensor_copy(
                            sb_oc[c][:, 128 * qb:128 * (qb + 1)],
                            tr[:, 128 * j:128 * (j + 1)])

            tr_pending.append(fin)

        def outproj(o, half):
            # half 0 rotates through ps_proj, half 1 through the (by now
            # idle) ps_s slots -> 4 psum slots total for the 16 chunks.
            if half == 0:
                ps = ps_proj.tile([128, BLK], f32, name="psy", tag="psq")
            else:
                ps = ps_s.tile([128, 1024], f32, name="psy", tag="psc")
            pw = ps[:, 0:256]
            for f in range(8):
                nc.tensor.matmul(pw, sb_wo[f][:, 128 * o:128 * (o + 1)],
                                 sb_oc[f][:, 256 * half:256 * (half + 1)],
                                 start=(f == 0), stop=(f == 7))
            yt = ytp.tile([128, 256], bf16, name="yt", tag="yt")
            if o % 2 == 0:
                nc.scalar.activation(yt[:], pw, Id, bias=sb_bo[:, o:o + 1])
            else:
                nc.vector.tensor_scalar_add(yt[:], pw, sb_bo[:, o:o + 1])
            nc.sync.dma_start(d_out[128 * o:128 * (o + 1), 256 * half:256 * (half + 1)], yt[:])

        # ---- emission schedule ------------------------------------------
        es = {}

        def s(c):
            es[c] = epool.tile([128, 3072], bf16, name="e", tag="e")
            scores(c, es[c])

        for c in range(8):
            qproj(c)
        for c in range(8):
            kproj(c)
        s(0)
        vproj(0, 0); vproj(0, 1)
        s(1)
        vproj(1, 0); vproj(1, 1)
        s(2)
        vproj(2, 0); vproj(2, 1)
        s(3)
        vproj(3, 0); vproj(3, 1)
        s(4)
        pv_half(0, 0, es[0])
        vproj(4, 0); vproj(4, 1)
        tr_flush()
        s(5)
        pv_half(1, 0, es[1])
        vproj(5, 0); vproj(5, 1)
        tr_flush()
        s(6)
        pv_half(0, 1, es[0])
        tr_flush()
        pv_half(2, 0, es[2])
        s(7)
        tr_flush()
        pv_half(1, 1, es[1])
        tr_flush()
        pv_half(3, 0, es[3])
        for c in range(4, 8):
            tr_flush()
            pv_half(c - 2, 1, es[c - 2])
            tr_flush()
            pv_half(c, 0, es[c])
        tr_flush()
        pv_half(6, 1, es[6])
        tr_flush()
        pv_half(7, 1, es[7])
        tr_flush()
        tr_flush()
        for half in range(2):
            for o in range(8):
                outproj(o, half)

    nc.compile()
    return nc


def _get_compiled():
    global _COMPILED
    if _COMPILED is None:
        _COMPILED = _build_bass()
    return _COMPILED


def kernel(x, Wq, bq, Wk, bk, Wv, bv, Wo, bo, _trace=False):
    from concourse.bass_utils import run_bass_kernel_spmd

    in_maps = _build_core_inputs(x, Wq, bq, Wk, bk, Wv, bv, Wo, bo)
    nc = _get_compiled()
    res = run_bass_kernel_spmd(nc, in_maps, core_ids=list(range(NCORES)),
                               trace=_trace)
    out = np.empty((B, S, H), np.float32)
    for c in range(NCORES):
        b, blk = divmod(c, 4)
        out[b, blk * BLK:(blk + 1) * BLK, :] = \
            res.results[c]["out"].astype(np.float32).T
    if _trace:
        return out, res
    return out


# revision 40
# speedup vs baseline: 1.1670x; 1.0238x over previous
"""LocalAttention (banded) Trainium2 kernel, 8-core SPMD.

Problem: B=2, S=2048, H=1024, nh=16, hd=64, window=256 (half_w=128).
  q = x@Wq+bq ; k = x@Wk+bk ; v = x@Wv+bv  (per-head dim 64)
  scores = q.k/8 masked to |i-j|<=128 ; out = softmax(scores)@v @ Wo + bo

Sharding: core c -> batch c//4, token block (c%4)*512..+512.  Each core
receives a zero-padded 768-token slice of x (128-token halo each side,
recomputed locally; no cross-core communication).

On-chip: fully "transposed" layout (features on partitions).  Scores are
computed transposed (S^T = K @ Q^T) in kc-major groups (wide moving
operands).  The softmax denominator is fused into the PV matmul by
appending a ones-column to each V tile; PV output is q-major so the
normalization is a per-partition broadcast multiply (no cross-partition
data movement anywhere).  The normalized q-major output chunk is
transposed back to feature-major via DMA-transpose on the sync/scalar
queues.  K/V projections and attention are interleaved on the tensor
queue; output projection is pipelined in q-halves at the end.
1/sqrt(hd) is folded into Wq/bq and bv@Wo+bo into a single output bias
on the host.
"""

import sys

if "/opt/trn_rl_repo" not in sys.path:
    sys.path.insert(0, "/opt/trn_rl_repo")

import numpy as np
import ml_dtypes

B, S, H = 2, 2048, 1024
NH, HD = 16, 64
HALF_W = 128
NCORES = 8
BLK = 512          # owned tokens per core
PAD = 768          # owned + 2*128 halo
NQB = 4            # q-blocks of 128 per core
NKC = 6            # padded-local k chunks of 128
BF16 = ml_dtypes.bfloat16

# kc-major score/e-tile layout: for key chunk kc, the valid q-blocks are
# qb in [KC_QB0[kc], KC_QB0[kc]+KC_NQB[kc]).  Column offset of chunk
# (kc, qb) inside the per-sub 1536-col half is
#   KC_OFF[kc] + 128*(qb - KC_QB0[kc]).
KC_QB0 = [0, 0, 0, 1, 2, 3]
KC_NQB = [1, 2, 3, 3, 2, 1]
KC_OFF = [0, 128, 384, 768, 1152, 1408]
GROUPS = [[0, 1], [2], [3], [4, 5]]     # kc's per 384-col score group
GLOC = {0: 0, 1: 128, 2: 0, 3: 0, 4: 0, 5: 256}  # chunk offset in group

_COMPILED = None


def _build_core_inputs(x, Wq, bq, Wk, bk, Wv, bv, Wo, bo):
    """Host-side sharding / layout prep. Returns list of 8 in_maps."""
    x = np.asarray(x, np.float32)
    scale = 1.0 / np.sqrt(HD)

    wq_s = (np.asarray(Wq, np.float32) * scale).astype(BF16)
    wk_s = np.asarray(Wk, np.float32).astype(BF16)
    wv_s = np.asarray(Wv, np.float32).astype(BF16)
    wo_s = np.asarray(Wo, np.float32).astype(BF16)
    bq_s = (np.asarray(bq, np.float32) * scale)
    bk_s = np.asarray(bk, np.float32)
    # v-bias passes through attention unchanged (softmax rows sum to 1),
    # so it folds into the output bias: bo' = bo + bv @ Wo.
    bo_s = np.asarray(bo, np.float32) + np.asarray(bv, np.float32) @ np.asarray(Wo, np.float32)

    def as_pcols(vec):  # [1024] -> [128, 8] with [:, c] = vec[128c:128c+128]
        return np.ascontiguousarray(vec.reshape(8, 128).T, dtype=np.float32)

    bq_t, bk_t, bo_t = as_pcols(bq_s), as_pcols(bk_s), as_pcols(bo_s)

    in_maps = []
    for c in range(NCORES):
        b, blk = divmod(c, 4)
        t0 = blk * BLK
        lo, hi = t0 - HALF_W, t0 + BLK + HALF_W
        xp = np.zeros((PAD, H), np.float32)
        glo, ghi = max(lo, 0), min(hi, S)
        xp[glo - lo:ghi - lo] = x[b, glo:ghi]
        xT = np.ascontiguousarray(xp.T, dtype=BF16)  # [1024, 768]

        # kc-major mask: tile element [p, KC_OFF[kc]+128*(qb-qb0)+i]
        # guards key token lo+128*kc+p vs query token t0+128*qb+i.
        mask = np.zeros((128, 1536), BF16)
        p = np.arange(128)
        for kc in range(NKC):
            kg = lo + 128 * kc + p
            for j in range(KC_NQB[kc]):
                qb = KC_QB0[kc] + j
                qg = t0 + 128 * qb + p
                valid = (np.abs(kg[:, None] - qg[None, :]) <= HALF_W) & \
                        (kg[:, None] >= 0) & (kg[:, None] < S)
                col = KC_OFF[kc] + 128 * j
                mask[:, col:col + 128] = valid
        in_maps.append({
            "xT": xT,
            "wq": wq_s, "wk": wk_s, "wv": wv_s, "wo": wo_s,
            "bq_t": bq_t, "bk_t": bk_t, "bo_t": bo_t,
            "mask": mask,
            "ident": np.eye(128, dtype=np.float32).astype(BF16),
        })
    return in_maps


def _build_bass():
    import concourse.bass as bass
    import concourse.tile as tile
    from concourse import bacc, mybir
    from contextlib import ExitStack

    f32, bf16 = mybir.dt.float32, mybir.dt.bfloat16
    Id = mybir.ActivationFunctionType.Identity
    Exp = mybir.ActivationFunctionType.Exp

    nc = bacc.Bacc(None)
    d_xT = nc.declare_dram_parameter("xT", [H, PAD], bf16, isOutput=False)
    d_wq = nc.declare_dram_parameter("wq", [H, H], bf16, isOutput=False)
    d_wk = nc.declare_dram_parameter("wk", [H, H], bf16, isOutput=False)
    d_wv = nc.declare_dram_parameter("wv", [H, H], bf16, isOutput=False)
    d_wo = nc.declare_dram_parameter("wo", [H, H], bf16, isOutput=False)
    d_bq = nc.declare_dram_parameter("bq_t", [128, 8], f32, isOutput=False)
    d_bk = nc.declare_dram_parameter("bk_t", [128, 8], f32, isOutput=False)
    d_bo = nc.declare_dram_parameter("bo_t", [128, 8], f32, isOutput=False)
    d_mask = nc.declare_dram_parameter("mask", [128, 1536], bf16, isOutput=False)
    d_ident = nc.declare_dram_parameter("ident", [128, 128], bf16, isOutput=False)
    d_out = nc.declare_dram_parameter("out", [H, BLK], bf16, isOutput=True)

    with tile.TileContext(nc) as tc, ExitStack() as ctx:
        persist = ctx.enter_context(tc.tile_pool(name="persist", bufs=1))
        epool = ctx.enter_context(tc.tile_pool(name="epool", bufs=8))
        smallp = ctx.enter_context(tc.tile_pool(name="smallp", bufs=3))
        ytp = ctx.enter_context(tc.tile_pool(name="ytp", bufs=4))
        ps_proj = ctx.enter_context(tc.tile_pool(name="ps_proj", bufs=2, space="PSUM"))
        ps_s = ctx.enter_context(tc.tile_pool(name="ps_s", bufs=2, space="PSUM"))
        ps_po = ctx.enter_context(tc.tile_pool(name="ps_po", bufs=2, space="PSUM"))

        sb_xT = [persist.tile([128, PAD], bf16, name=f"xT{h}", tag=f"xT{h}") for h in range(8)]
        sb_wq = [persist.tile([128, H], bf16, name=f"wq{h}", tag=f"wq{h}") for h in range(8)]
        sb_wk = [persist.tile([128, H], bf16, name=f"wk{h}", tag=f"wk{h}") for h in range(8)]
        sb_wv = [persist.tile([128, H], bf16, name=f"wv{h}", tag=f"wv{h}") for h in range(8)]
        sb_wo = [persist.tile([128, H], bf16, name=f"wo{h}", tag=f"wo{h}") for h in range(8)]
        sb_qt = [persist.tile([128, BLK], bf16, name=f"qt{c}", tag=f"qt{c}") for c in range(8)]
        sb_kt = [persist.tile([128, PAD], bf16, name=f"kt{c}", tag=f"kt{c}") for c in range(8)]
        # v token-major, per head-pair column groups of 130:
        #   [64 f sub0][one][64 f sub1][one]
        sb_v = [persist.tile([128, 1040], bf16, name=f"v{t}", tag=f"v{t}") for t in range(6)]
        sb_oc = [persist.tile([128, BLK], bf16, name=f"oc{c}", tag=f"oc{c}") for c in range(8)]
        sb_mask = persist.tile([128, 1536], bf16, name="mask", tag="mask")
        sb_bq = persist.tile([128, 8], f32, name="bq", tag="bq")
        sb_bk = persist.tile([128, 8], f32, name="bk", tag="bk")
        sb_bo = persist.tile([128, 8], f32, name="bo", tag="bo")
        sb_ident = persist.tile([128, 128], bf16, name="ident", tag="ident")

        # ---- DMA issue --------------------------------------------------
        # gpsimd's software DGE issues back-to-back (~650ns/256KB) while
        # the HWDGE rings (sync/scalar) only sustain ~1 transfer/1.3us, so
        # the start-gating xT/wq pairs go mostly on gpsimd.
        for h in range(4):
            nc.gpsimd.dma_start(sb_xT[h][:], d_xT[128 * h:128 * (h + 1), :])
            nc.gpsimd.dma_start(sb_wq[h][:], d_wq[128 * h:128 * (h + 1), :])
        for h in range(4, 6):
            nc.sync.dma_start(sb_xT[h][:], d_xT[128 * h:128 * (h + 1), :])
            nc.sync.dma_start(sb_wq[h][:], d_wq[128 * h:128 * (h + 1), :])
        for h in range(6, 8):
            nc.scalar.dma_start(sb_xT[h][:], d_xT[128 * h:128 * (h + 1), :])
            nc.scalar.dma_start(sb_wq[h][:], d_wq[128 * h:128 * (h + 1), :])
        nc.scalar.dma_start(sb_bq[:], d_bq[:])
        nc.scalar.dma_start(sb_bk[:], d_bk[:])
        nc.scalar.dma_start(sb_bo[:], d_bo[:])
        nc.scalar.dma_start(sb_ident[:], d_ident[:])
        for h in range(8):
            nc.sync.dma_start(sb_wk[h][:], d_wk[128 * h:128 * (h + 1), :])
        for h in range(8):
            nc.gpsimd.dma_start(sb_wv[h][:], d_wv[128 * h:128 * (h + 1), :])
        for h in range(8):
            nc.sync.dma_start(sb_wo[h][:], d_wo[128 * h:128 * (h + 1), :])
        nc.gpsimd.dma_start(sb_mask[:], d_mask[:])

        # ones columns of the v tiles (cols 130c+64 and 130c+129)
        for t in range(6):
            vv = sb_v[t][:]
            ones_ap = bass.AP(tensor=vv.tensor, offset=vv.offset + 64,
                              ap=[vv.ap[0], [130, 8], [65, 2]])
            nc.vector.memset(ones_ap, 1.0)

        # ---- compute emitters -------------------------------------------
        def qproj(c):
            ps = ps_proj.tile([128, BLK], f32, name="psq", tag="psq")
            for h in range(8):
                nc.tensor.matmul(ps[:], sb_wq[h][:, 128 * c:128 * (c + 1)],
                                 sb_xT[h][:, 128:128 + BLK],
                                 start=(h == 0), stop=(h == 7))
            nc.scalar.activation(sb_qt[c][:], ps[:], Id, bias=sb_bq[:, c:c + 1])

        def kproj(c):
            for half in range(2):
                ps = ps_proj.tile([128, BLK], f32, name="psk", tag="psq")
                pv_ = ps[:, 0:384]
                for h in range(8):
                    nc.tensor.matmul(pv_, sb_wk[h][:, 128 * c:128 * (c + 1)],
                                     sb_xT[h][:, 384 * half:384 * (half + 1)],
                                     start=(h == 0), stop=(h == 7))
                nc.vector.tensor_scalar_add(
                    sb_kt[c][:, 384 * half:384 * (half + 1)], pv_,
                    sb_bk[:, c:c + 1])

        def vproj(t, half):
            ps = ps_proj.tile([128, BLK], f32, name="psv", tag="psq")
            for h in range(8):
                nc.tensor.matmul(ps[:], sb_xT[h][:, 128 * t:128 * (t + 1)],
                                 sb_wv[h][:, 512 * half:512 * (half + 1)],
                                 start=(h == 0), stop=(h == 7))
            vv = sb_v[t][:]
            dst = bass.AP(tensor=vv.tensor, offset=vv.offset + 520 * half,
                          ap=[vv.ap[0], [130, 4], [65, 2], [1, 64]])
            nc.vector.tensor_copy(dst, ps[:])

        def scores(c, e):
            ee = e[:]
            mm_ = sb_mask[:]
            for g in range(4):
                psc = ps_s.tile([128, 1024], f32, name="psc", tag="psc")
                pp = psc[:]
                for sub in range(2):
                    for kc in GROUPS[g]:
                        qb0 = KC_QB0[kc]
                        W = 128 * KC_NQB[kc]
                        loc = 512 * sub + GLOC[kc]
                        nc.tensor.matmul(
                            psc[:, loc:loc + W],
                            sb_kt[c][64 * sub:64 * (sub + 1), 128 * kc:128 * (kc + 1)],
                            sb_qt[c][64 * sub:64 * (sub + 1), 128 * qb0:128 * qb0 + W])
                src = bass.AP(tensor=pp.tensor, offset=pp.offset,
                              ap=[pp.ap[0], [512, 2], [1, 384]])
                dst = bass.AP(tensor=ee.tensor, offset=ee.offset + 384 * g,
                              ap=[ee.ap[0], [1536, 2], [1, 384]])
                nc.scalar.activation(dst, src, Exp)
                # mask the two triangle chunks of this group (both subs),
                # alternating engines (e and mask are SBUF so gpsimd is legal)
                ev = bass.AP(tensor=ee.tensor, offset=ee.offset + 384 * g,
                             ap=[ee.ap[0], [1536, 2], [256, 2], [1, 128]])
                mv = bass.AP(tensor=mm_.tensor, offset=mm_.offset + 384 * g,
                             ap=[mm_.ap[0], [0, 2], [256, 2], [1, 128]])
                eng = nc.gpsimd if g == 3 else nc.vector
                eng.tensor_mul(ev, ev, mv)

        # PE transposes of the normalized q-major chunk back to feature-
        # major are emitted one pv_half later (the normalize on vector must
        # land first; the lag keeps the in-order tensor queue stall-free).
        tr_pending = []

        def tr_flush():
            if tr_pending:
                tr_pending.pop(0)()

        def pv_half(c, half, e):
            # po layout: [128 q, 512]: region (j, sub) at 130*j + 65*sub;
            # 65 cols = 64 features + denominator.  half covers qb 2h..2h+1.
            ocq = smallp.tile([128, 256], bf16, name="ocq", tag="ocq")
            po = ps_po.tile([128, BLK], f32, name="po", tag="po")
            pp = po[:]
            for j in range(2):
                qb = 2 * half + j
                for sub in range(2):
                    for r in range(3):
                        kc = qb + r
                        ecol = 1536 * sub + KC_OFF[kc] + 128 * (qb - KC_QB0[kc])
                        nc.tensor.matmul(
                            po[:, 130 * j + 65 * sub:130 * j + 65 * (sub + 1)],
                            e[:, ecol:ecol + 128],
                            sb_v[kc][:, 130 * c + 65 * sub:130 * c + 65 * (sub + 1)],
                            start=(r == 0), stop=(r == 2))
            tr_flush()
            rc = smallp.tile([128, 4], f32, name="rc", tag="rc")
            dsrc = bass.AP(tensor=pp.tensor, offset=pp.offset + 64,
                           ap=[pp.ap[0], [65, 4]])
            nc.vector.reciprocal(rc[:], dsrc)
            for sub in range(2):
                src = bass.AP(tensor=pp.tensor, offset=pp.offset + 65 * sub,
                              ap=[pp.ap[0], [130, 2], [1, 64]])
                rv = bass.AP(tensor=rc.tensor, offset=rc[:].offset + sub,
                             ap=[rc[:].ap[0], [2, 2], [0, 64]])
                ov = bass.AP(tensor=ocq.tensor,
                             offset=ocq[:].offset + 64 * sub,
                             ap=[ocq[:].ap[0], [128, 2], [1, 64]])
                nc.vector.tensor_mul(ov, src, rv)

            def fin():
                # bf16 view-sized tile sharing the psc slot byte size
                tr = ps_s.tile([128, 2048], bf16, name="tr", tag="psc")
                for j in range(2):
                    nc.tensor.transpose(tr[:, 128 * j:128 * (j + 1)],
                                        ocq[:, 128 * j:128 * (j + 1)],
                                        sb_ident[:])
                for j in range(2):
                    qb = 2 * half + j
                    nc.vector.tensor_copy(
                        sb_oc[c][:, 128 * qb:128 * (qb + 1)],
                        tr[:, 128 * j:128 * (j + 1)])

            tr_pending.append(fin)

        def outproj(o, half):
            # half 0 rotates through ps_proj, half 1 through the (by now
            # idle) ps_s slots -> 4 psum slots total for the 16 chunks.
            if half == 0:
                ps = ps_proj.tile([128, BLK], f32, name="psy", tag="psq")
            else:
                ps = ps_s.tile([128, 1024], f32, name="psy", tag="psc")
            pw = ps[:, 0:256]
            for f in range(8):
                nc.tensor.matmul(pw, sb_wo[f][:, 128 * o:128 * (o + 1)],
                                 sb_oc[f][:, 256 * half:256 * (half + 1)],
                                 start=(f == 0), stop=(f == 7))
            yt = ytp.tile([128, 256], bf16, name="yt", tag="yt")
            nc.scalar.activation(yt[:], pw, Id, bias=sb_bo[:, o:o + 1])
            nc.sync.dma_start(d_out[128 * o:128 * (o + 1), 256 * half:256 * (half + 1)], yt[:])

        # ---- emission schedule ------------------------------------------
        es = {}

        def s(c):
            es[c] = epool.tile([128, 3072], bf16, name="e", tag="e")
            scores(c, es[c])

        for c in range(8):
            qproj(c)
        for c in range(8):
            kproj(c)
        s(0)
        vproj(0, 0); vproj(0, 1)
        s(1)
        vproj(1, 0); vproj(1, 1)
        s(2)
        vproj(2, 0); vproj(2, 1)
        s(3)
        vproj(3, 0); vproj(3, 1)
        s(4)
        pv_half(0, 0, es[0])
        vproj(4, 0); vproj(4, 1)
        s(5)
        pv_half(1, 0, es[1])
        vproj(5, 0); vproj(5, 1)
        s(6)
        pv_half(2, 0, es[2])
        s(7)
        pv_half(3, 0, es[3])
        for c in range(4, 8):
            pv_half(c, 0, es[c])
        for c in range(8):
            pv_half(c, 1, es[c])
        tr_flush()
        for half in range(2):
            for o in range(8):
                outproj(o, half)

    nc.compile()
    return nc


def _get_compiled():
    global _COMPILED
    if _COMPILED is None:
        _COMPILED = _build_bass()
    return _COMPILED


def kernel(x, Wq, bq, Wk, bk, Wv, bv, Wo, bo, _trace=False):
    from concourse.bass_utils import run_bass_kernel_spmd

    in_maps = _build_core_inputs(x, Wq, bq, Wk, bk, Wv, bv, Wo, bo)
    nc = _get_compiled()
    res = run_bass_kernel_spmd(nc, in_maps, core_ids=list(range(NCORES)),
                               trace=_trace)
    out = np.empty((B, S, H), np.float32)
    for c in range(NCORES):
        b, blk = divmod(c, 4)
        out[b, blk * BLK:(blk + 1) * BLK, :] = \
            res.results[c]["out"].astype(np.float32).T
    if _trace:
        return out, res
    return out
